# revision 23
# baseline (speedup 1.0000x reference)
"""BiMamba block Trainium2 kernel (v2).

Sharding (8 cores): core = b*4 + dir*2 + dh
  b   in {0,1}: batch element
  dir in {0,1}: scan direction (0=forward, 1=backward). Backward cores
                receive the token stream reversed by the host, so the
                device program is direction-agnostic (pure SPMD).
  dh  in {0,1}: half of d_inner (tensor-parallel over channels).

Device collectives:
  x_dbl AllReduce over dh-pairs       [[0,1],[2,3],[4,5],[6,7]]
  y AllGather over dir-pairs          [[0,2],[1,3],[4,6],[5,7]]
  out partial ReduceScatter, dh-pairs [[0,1],[2,3],[4,5],[6,7]]

Scan phase (P2) structure per core: k-tile outer, time-half middle,
state n inner.
  - B/C rows are broadcast to 128 partitions by partition-stride-0 DMA
    from DRAM (no matmul + PSUM + scalar-copy).
  - The sum over the 16 states runs on the tensor engine as
    identity-matmul accumulation into PSUM (replacing vector/gpsimd
    tensor adds).
  - The h*C product runs on gpsimd; the vector engine keeps only the
    B-mult and the scan itself.
Output partials are kept in transposed [e, t] layout end-to-end (the
ReduceScatter is elementwise, the host transposes shards), which
removes all P3 transposes.
"""

import numpy as np
import ml_dtypes

import concourse.bass as bass
import concourse.mybir as mybir
import concourse.tile as tile
from concourse import bacc, bass_utils

F32 = mybir.dt.float32
BF16 = mybir.dt.bfloat16
AF = mybir.ActivationFunctionType
ALU = mybir.AluOpType


class Cfg:
    def __init__(self, L=4096, DM=1024, DI=2048, DTR=64, DS=16, DCONV=4,
                 NB=2, EPS=1e-5):
        self.L = L          # sequence length (per stream)
        self.DM = DM        # d_model
        self.DI = DI        # d_inner
        self.DLOC = DI // 2  # channels per core
        self.DTR = DTR      # dt_rank
        self.DS = DS        # d_state
        self.DCONV = DCONV
        self.NB = NB        # batch elements
        self.EPS = EPS
        self.NXP = DTR + 2 * DS    # x_proj output dim
        self.EOUT = DM // 2        # output columns per core
        self.NCORES = NB * 4
        self.KT = self.DLOC // 128   # d-tiles per core (8)
        self.CT = DM // 128          # channel tiles of x (8)
        self.MT = 2 * self.DLOC // 128  # in_proj output tiles (16)
        self.NBLK = L // 512         # 512-token blocks (8)
        self.THL = L // 2            # scan t-half length (2048)
        self.ER = self.EOUT // 2     # output e-rows per core (256)
        # groups
        self.g_dh = [[b * 4 + d * 2, b * 4 + d * 2 + 1]
                     for b in range(NB) for d in range(2)]
        self.g_dir = [[b * 4 + dh, b * 4 + 2 + dh]
                      for b in range(NB) for dh in range(2)]


def build_program(cfg: Cfg):
    c = cfg
    nc = bacc.Bacc("TRN2", num_devices=c.NCORES)

    # ---------------- I/O ----------------
    x_in = nc.dram_tensor("x", [c.L, c.DM], F32, kind="ExternalInput")
    win_t = nc.dram_tensor("win_t", [c.DM, 2 * c.DLOC], BF16, kind="ExternalInput")
    wxp_t = nc.dram_tensor("wxp_t", [c.DLOC, c.NXP], BF16, kind="ExternalInput")
    wdt_t = nc.dram_tensor("wdt_t", [c.DTR, c.DLOC], BF16, kind="ExternalInput")
    dtb = nc.dram_tensor("dtb", [c.DLOC, 1], F32, kind="ExternalInput")
    convw = nc.dram_tensor("convw", [c.DLOC, c.DCONV], F32, kind="ExternalInput")
    convb = nc.dram_tensor("convb", [c.DLOC, 1], F32, kind="ExternalInput")
    arow = nc.dram_tensor("arow", [1, c.DS], F32, kind="ExternalInput")
    dvec = nc.dram_tensor("dvec", [c.DLOC, 1], F32, kind="ExternalInput")
    wout_t = nc.dram_tensor("wout_t", [c.DLOC, c.EOUT], BF16, kind="ExternalInput")
    xres = nc.dram_tensor("xres", [c.ER, c.L], F32, kind="ExternalInput")
    out = nc.dram_tensor("out", [c.ER, c.L], F32, kind="ExternalOutput")

    # ---------------- DRAM scratch ----------------
    xi_st = nc.dram_tensor("xi_st", [c.DLOC, c.L], F32)
    xc_st = nc.dram_tensor("xc_st", [c.DLOC, c.L], BF16)
    z_st = nc.dram_tensor("z_st", [c.DLOC, c.L], BF16)
    t1_st = nc.dram_tensor("t1_st", [c.DLOC, c.L], BF16)   # silu(z)
    xd_in = nc.dram_tensor("xd_in", [c.NXP, c.L], BF16)
    xd_out = nc.dram_tensor("xd_out", [c.NXP, c.L], BF16)
    y_in = [nc.dram_tensor(f"y_in{th}", [c.DLOC, c.THL], BF16)
            for th in range(2)]
    ycc_st = nc.dram_tensor("ycc_st", [c.DLOC, c.L], BF16)
    y_agp = [[nc.dram_tensor(f"y_agp{i}_{th}", [2 * 128, c.THL], BF16)
              for th in range(2)] for i in range(c.KT)]
    rs_in = nc.dram_tensor("rs_in", [c.EOUT, c.L], BF16)
    rs_out = nc.dram_tensor("rs_out", [c.ER, c.L], BF16)

    def rev_ap(t, n):
        """AP reading AP/tile t with the free (last) dim reversed (length n)."""
        a = t[:] if hasattr(t, 'tile_id') or not isinstance(t, bass.AP) else t
        ap = [list(d) for d in a.ap]
        assert ap[-1][0] == 1 and ap[-1][1] == n
        ap[-1] = [-1, n]
        return bass.AP(tensor=a.tensor, offset=a.offset + (n - 1), ap=ap)

    with tile.TileContext(nc) as tc:
        # ======== persistent constants ========
        with tc.tile_pool(name="wts", bufs=1) as wts:
            ident = wts.tile([128, 128], F32, tag="ident", name="ident")
            from concourse.masks import make_identity
            make_identity(nc, ident[:])
            identb = wts.tile([128, 128], BF16, tag="identb", name="identb")
            nc.vector.tensor_copy(identb[:], ident[:])
            eps_c = wts.tile([128, 1], F32, tag="eps_c", name="eps_c")
            nc.vector.memset(eps_c[:], c.EPS)

            acols = []
            for n in range(c.DS):
                acol = wts.tile([128, 1], F32, tag=f"acol{n}", name=f"acol{n}")
                nc.sync.dma_start(
                    out=acol[:],
                    in_=bass.AP(tensor=arow, offset=n, ap=[[0, 128], [1, 1]]))
                acols.append(acol)

            dtb_c, dv_c, cw_c, cb_c = [], [], [], []
            for k in range(c.KT):
                t1 = wts.tile([128, 1], F32, tag=f"dtb{k}", name=f"dtb{k}")
                nc.sync.dma_start(out=t1[:], in_=dtb[k * 128:(k + 1) * 128, :])
                dtb_c.append(t1)
                t2 = wts.tile([128, 1], F32, tag=f"dv{k}", name=f"dv{k}")
                nc.sync.dma_start(out=t2[:], in_=dvec[k * 128:(k + 1) * 128, :])
                dv_c.append(t2)
                t3 = wts.tile([128, c.DCONV], F32, tag=f"cw{k}", name=f"cw{k}")
                nc.sync.dma_start(out=t3[:], in_=convw[k * 128:(k + 1) * 128, :])
                cw_c.append(t3)
                t4 = wts.tile([128, 1], F32, tag=f"cb{k}", name=f"cb{k}")
                nc.sync.dma_start(out=t4[:], in_=convb[k * 128:(k + 1) * 128, :])
                cb_c.append(t4)

            # ======== P0: norm + transpose + in_proj ========
            with tc.tile_pool(name="p0w", bufs=1) as p0w, \
                 tc.tile_pool(name="p0", bufs=3) as p0, \
                 tc.tile_pool(name="p0t", bufs=1) as p0t, \
                 tc.tile_pool(name="p0ps", bufs=2, space="PSUM") as p0ps, \
                 tc.tile_pool(name="p0pm", bufs=4, space="PSUM") as p0pm:
                win_sb = []
                for k2 in range(c.CT):
                    w = p0w.tile([128, 2 * c.DLOC], BF16, tag=f"win{k2}", name=f"win{k2}")
                    nc.sync.dma_start(out=w[:],
                                      in_=win_t[k2 * 128:(k2 + 1) * 128, :])
                    win_sb.append(w)

                xnT_all = {}
                for tb in range(c.NBLK):
                    xnT = [p0t.tile([128, 512], BF16, tag=f"xnT{tb}_{k2}", name=f"xnT{tb}_{k2}")
                           for k2 in range(c.CT)]
                    xnT_all[tb] = xnT
                    for tt in range(4):
                        rows = slice(tb * 512 + tt * 128,
                                     tb * 512 + (tt + 1) * 128)
                        xt = p0.tile([128, c.DM], F32, tag="xt", name="xt")
                        nc.sync.dma_start(out=xt[:], in_=x_in[rows, :])
                        xsq = p0.tile([128, c.DM], F32, tag="xsq", name="xsq")
                        ssc = p0.tile([128, 1], F32, tag="ssc", name="ssc")
                        nc.scalar.activation(xsq[:], xt[:], AF.Square,
                                             accum_out=ssc[:])
                        sq = p0.tile([128, 1], F32, tag="sq", name="sq")
                        nc.scalar.activation(sq[:], ssc[:], AF.Sqrt,
                                             scale=1.0 / c.DM, bias=eps_c[:])
                        rn = p0.tile([128, 1], F32, tag="rn", name="rn")
                        nc.vector.reciprocal(rn[:], sq[:])
                        xn = p0.tile([128, c.DM], F32, tag="xn", name="xn")
                        nc.vector.tensor_scalar_mul(xn[:], xt[:], rn[:])
                        for ct4 in range(max(1, c.CT // 4)):
                            nsub = min(4, c.CT - ct4 * 4)
                            pst = p0ps.tile([128, 512], F32, tag="pst", name="pst")
                            for j in range(nsub):
                                ct = ct4 * 4 + j
                                nc.tensor.transpose(
                                    pst[:, j * 128:(j + 1) * 128],
                                    xn[:, ct * 128:(ct + 1) * 128], ident[:])
                            for j in range(nsub):
                                ct = ct4 * 4 + j
                                nc.scalar.activation(
                                    xnT[ct][:, tt * 128:(tt + 1) * 128],
                                    pst[:, j * 128:(j + 1) * 128], AF.Copy)
                for m in range(c.MT):
                    for tb in range(c.NBLK):
                        ps = p0pm.tile([128, 512], F32, tag="mm", name="mm")
                        for k2 in range(c.CT):
                            nc.tensor.matmul(
                                ps[:],
                                win_sb[k2][:, m * 128:(m + 1) * 128],
                                xnT_all[tb][k2][:],
                                start=(k2 == 0), stop=(k2 == c.CT - 1))
                        if m < c.KT:
                            dst, r0 = xi_st, m * 128
                            pcp = p0.tile([128, 512], F32, tag="pcp", name="pcp")
                        else:
                            dst, r0 = z_st, (m - c.KT) * 128
                            pcp = p0.tile([128, 512], BF16, tag="pcpb", name="pcpb")
                        nc.vector.tensor_copy(pcp[:], ps[:])
                        nc.sync.dma_start(
                            out=dst[r0:r0 + 128, tb * 512:(tb + 1) * 512],
                            in_=pcp[:])

            # ======== P1: conv + silu + x_proj partials; silu(z) prep ======
            with tc.tile_pool(name="p1", bufs=2) as p1, \
                 tc.tile_pool(name="p1ps", bufs=1, space="PSUM") as p1ps:
                xdp = [p1ps.tile([c.NXP, 512], F32, tag=f"xdp{nb}", name=f"xdp{nb}")
                       for nb in range(c.NBLK)]
                for k in range(c.KT):
                    xi = p1.tile([128, c.L], F32, tag="xi", name="xi")
                    nc.sync.dma_start(out=xi[:],
                                      in_=xi_st[k * 128:(k + 1) * 128, :])
                    cv = p1.tile([128, c.L], F32, tag="cv", name="cv")
                    nc.vector.tensor_scalar_mul(cv[:], xi[:], cw_c[k][:, 3:4])
                    for kk in (2, 1, 0):
                        sh = 3 - kk
                        nc.vector.scalar_tensor_tensor(
                            cv[:, sh:c.L], xi[:, 0:c.L - sh],
                            cw_c[k][:, kk:kk + 1],
                            cv[:, sh:c.L], ALU.mult, ALU.add)
                    nc.vector.tensor_scalar_add(cv[:], cv[:], cb_c[k][:])
                    sg = p1.tile([128, c.L], F32, tag="sg", name="sg")
                    nc.scalar.activation(sg[:], cv[:], AF.Sigmoid)
                    xcb = p1.tile([128, c.L], BF16, tag="xcb", name="xcb")
                    nc.vector.tensor_tensor(xcb[:], cv[:], sg[:], op=ALU.mult)
                    nc.sync.dma_start(out=xc_st[k * 128:(k + 1) * 128, :],
                                      in_=xcb[:])
                    wxp = p1.tile([128, c.NXP], BF16, tag="wxp", name="wxp")
                    nc.sync.dma_start(out=wxp[:],
                                      in_=wxp_t[k * 128:(k + 1) * 128, :])
                    for nb in range(c.NBLK):
                        nc.tensor.matmul(
                            xdp[nb][:], wxp[:],
                            xcb[:, nb * 512:(nb + 1) * 512],
                            start=(k == 0), stop=(k == c.KT - 1))
                for nb in range(c.NBLK):
                    xdc = p1.tile([c.NXP, 512], BF16, tag="xdc", name="xdc")
                    nc.vector.tensor_copy(xdc[:], xdp[nb][:])
                    nc.sync.dma_start(
                        out=xd_in[:, nb * 512:(nb + 1) * 512], in_=xdc[:])

            nc.gpsimd.collective_compute(
                "AllReduce", ALU.add, ins=[xd_in.ap()], outs=[xd_out.ap()],
                replica_groups=c.g_dh)

            # silu(z) gate prep — independent of the AR, fills its latency
            with tc.tile_pool(name="p1z", bufs=2) as p1z:
                for k in range(c.KT):
                    zb = p1z.tile([128, c.L], BF16, tag="zb", name="zb")
                    nc.sync.dma_start(out=zb[:],
                                      in_=z_st[k * 128:(k + 1) * 128, :])
                    sgz = p1z.tile([128, c.L], BF16, tag="sgz", name="sgz")
                    nc.scalar.activation(sgz[:], zb[:], AF.Sigmoid)
                    t1k = p1z.tile([128, c.L], BF16, tag="t1k", name="t1k")
                    nc.vector.tensor_tensor(t1k[:], sgz[:], zb[:], op=ALU.mult)
                    nc.sync.dma_start(out=t1_st[k * 128:(k + 1) * 128, :],
                                      in_=t1k[:])

            # ======== P2: dt_proj + scan core ========
            with tc.tile_pool(name="p2w", bufs=1) as p2w:
                xdbl = p2w.tile([c.DTR, c.L], BF16, tag="xdbl", name="xdbl")
                nc.sync.dma_start(out=xdbl[:], in_=xd_out[0:c.DTR, :])
                wdt = p2w.tile([c.DTR, c.DLOC], BF16, tag="wdt", name="wdt")
                nc.sync.dma_start(out=wdt[:], in_=wdt_t[:, :])

                with tc.tile_pool(name="p2k2", bufs=2) as p2k2, \
                     tc.tile_pool(name="p2k1", bufs=1) as p2k1, \
                     tc.tile_pool(name="p2r", bufs=3) as p2r, \
                     tc.tile_pool(name="p2s", bufs=2) as p2s, \
                     tc.tile_pool(name="p2h", bufs=2) as p2h, \
                     tc.tile_pool(name="p2hi", bufs=1) as p2hi, \
                     tc.tile_pool(name="p2g", bufs=1) as p2g, \
                     tc.tile_pool(name="p2dps", bufs=2, space="PSUM") as p2dps, \
                     tc.tile_pool(name="p2ya", bufs=1, space="PSUM") as p2ya:
                    hinit = [p2hi.tile([128, 1], BF16, tag=f"hi{n}", name=f"hi{n}")
                             for n in range(c.DS)]

                    def emit_an(dl, th, n):
                        an = p2s.tile([128, c.THL], F32, tag="an", name="an")
                        nc.scalar.activation(
                            an[:], dl[:, th * c.THL:(th + 1) * c.THL],
                            AF.Exp, scale=acols[n][:])
                        return an

                    def dt_chain(k):
                        """dl/du/xcd for k-tile k (software-pipelined)."""
                        dl = p2k2.tile([128, c.L], F32, tag="dl", name="dl")
                        for nb in range(c.NBLK):
                            dps = p2dps.tile([128, 512], F32, tag="dps", name="dps")
                            nc.tensor.matmul(
                                dps[:],
                                wdt[:, k * 128:(k + 1) * 128],
                                xdbl[0:c.DTR, nb * 512:(nb + 1) * 512],
                                start=True, stop=True)
                            esl = p2k1.tile([128, 512], F32, tag="esl", name="esl")
                            nc.scalar.activation(esl[:], dps[:], AF.Exp,
                                                 bias=dtb_c[k][:])
                            nc.scalar.activation(
                                dl[:, nb * 512:(nb + 1) * 512], esl[:],
                                AF.Ln, bias=1.0)
                        xcb = p2k1.tile([128, c.L], BF16, tag="xck", name="xck")
                        nc.sync.dma_start(
                            out=xcb[:], in_=xc_st[k * 128:(k + 1) * 128, :])
                        du = p2k2.tile([128, c.L], BF16, tag="du", name="du")
                        nc.vector.tensor_tensor(du[:], dl[:], xcb[:],
                                                op=ALU.mult)
                        xcd = p2k2.tile([128, c.L], BF16, tag="xcd", name="xcd")
                        nc.vector.tensor_scalar_mul(xcd[:], xcb[:], dv_c[k][:])
                        return dl, du, xcd

                    def combine_dir(k):
                        """ycc = y_f + rev(y_b) for k-tile k (both halves)."""
                        yk = p2g.tile([128, c.L], BF16, tag="ycmb", name="ycmb")
                        for th in range(2):
                            b0 = p2g.tile([128, c.THL], BF16, tag="b0", name="b0")
                            nc.sync.dma_start(out=b0[:],
                                              in_=y_agp[k][th][0:128, :])
                            b1 = p2g.tile([128, c.THL], BF16, tag="b1", name="b1")
                            nc.sync.dma_start(out=b1[:],
                                              in_=y_agp[k][1 - th][128:256, :])
                            nc.vector.tensor_tensor(
                                yk[:, th * c.THL:(th + 1) * c.THL], b0[:],
                                rev_ap(b1[:], c.THL), op=ALU.add)
                        nc.sync.dma_start(
                            out=ycc_st[k * 128:(k + 1) * 128, :], in_=yk[:])

                    cur = dt_chain(0)
                    an_next = emit_an(cur[0], 0, 0)
                    for k in range(c.KT):
                        dl, du, xcd = cur
                        ya_sb = p2k1.tile([128, c.L], BF16, tag="yasb", name="yasb")
                        t1k = p2k1.tile([128, c.L], BF16, tag="t1g", name="t1g")
                        nc.sync.dma_start(
                            out=t1k[:], in_=t1_st[k * 128:(k + 1) * 128, :])
                        for th in range(2):
                            t0 = th * c.THL
                            tsl = slice(t0, t0 + c.THL)
                            ya_ps = p2ya.tile([128, c.THL], F32, tag="ya", name="ya")
                            # init ya with D*xc (identity matmul)
                            for j in range(c.THL // 512):
                                nc.tensor.matmul(
                                    ya_ps[:, j * 512:(j + 1) * 512], identb[:],
                                    xcd[:, t0 + j * 512:t0 + (j + 1) * 512],
                                    start=True, stop=False)
                            for n in range(c.DS):
                                brow = p2r.tile([128, c.THL], BF16,
                                                tag="brow", name="brow")
                                nc.sync.dma_start(
                                    out=brow[:],
                                    in_=bass.AP(tensor=xd_out,
                                                offset=(c.DTR + n) * c.L + t0,
                                                ap=[[0, 128], [1, c.THL]]))
                                crow = p2r.tile([128, c.THL], BF16,
                                                tag="crow", name="crow")
                                nc.sync.dma_start(
                                    out=crow[:],
                                    in_=bass.AP(tensor=xd_out,
                                                offset=(c.DTR + c.DS + n) * c.L + t0,
                                                ap=[[0, 128], [1, c.THL]]))
                                bn = p2s.tile([128, c.THL], BF16, tag="bn", name="bn")
                                nc.vector.tensor_tensor(bn[:], du[:, tsl],
                                                        brow[:], op=ALU.mult)
                                an_cur = an_next
                                if n + 1 < c.DS:
                                    an_next = emit_an(dl, th, n + 1)
                                elif th == 0:
                                    an_next = emit_an(dl, 1, 0)
                                h = p2h.tile([128, c.THL], BF16, tag="h", name="h")
                                init = 0.0 if th == 0 else hinit[n][:, 0:1]
                                nc.vector.tensor_tensor_scan(
                                    h[:], an_cur[:], bn[:], init,
                                    ALU.mult, ALU.add)
                                if th == 0:
                                    nc.scalar.activation(
                                        hinit[n][:], h[:, c.THL - 1:c.THL],
                                        AF.Copy)
                                zt = p2s.tile([128, c.THL], BF16, tag="zt", name="zt")
                                nc.vector.tensor_tensor(zt[:], h[:], crow[:],
                                                        op=ALU.mult)
                                for j in range(c.THL // 512):
                                    nc.tensor.matmul(
                                        ya_ps[:, j * 512:(j + 1) * 512],
                                        identb[:],
                                        zt[:, j * 512:(j + 1) * 512],
                                        start=False, stop=(n == c.DS - 1))
                            # next k's dt chain while th1 scans
                            if th == 0 and k + 1 < c.KT:
                                cur = dt_chain(k + 1)
                            nc.scalar.activation(ya_sb[:, tsl], ya_ps[:],
                                                 AF.Copy)
                            # per-half gate + AllGather
                            ycg = p2s.tile([128, c.THL], BF16, tag="ycg", name="ycg")
                            nc.vector.tensor_tensor(ycg[:], t1k[:, tsl],
                                                    ya_sb[:, tsl],
                                                    op=ALU.mult)
                            nc.sync.dma_start(
                                out=y_in[th][k * 128:(k + 1) * 128, :],
                                in_=ycg[:])
                            nc.gpsimd.collective_compute(
                                "AllGather", ALU.bypass,
                                ins=[y_in[th][k * 128:(k + 1) * 128, :]],
                                outs=[y_agp[k][th].ap()],
                                replica_groups=c.g_dir)
                        if k + 1 < c.KT:
                            an_next = emit_an(cur[0], 0, 0)
                        # combine directions for k-2 (its AllGathers landed
                        # long ago -> no vector stall): ycc = y_f + rev(y_b)
                        if k >= 2:
                            combine_dir(k - 2)
                    combine_dir(c.KT - 2)
                    combine_dir(c.KT - 1)

            # ======== P3: out_proj (transposed layout, chunked RS) ======
            with tc.tile_pool(name="p3w", bufs=1) as p3w, \
                 tc.tile_pool(name="p3y", bufs=1) as p3y, \
                 tc.tile_pool(name="p3o", bufs=3) as p3o, \
                 tc.tile_pool(name="p4", bufs=2) as p4, \
                 tc.tile_pool(name="p3ps", bufs=4, space="PSUM") as p3ps:
                wout_sb = []
                for k in range(c.KT):
                    w = p3w.tile([128, c.EOUT], BF16, tag=f"wo{k}", name=f"wo{k}")
                    nc.sync.dma_start(out=w[:],
                                      in_=wout_t[k * 128:(k + 1) * 128, :])
                    wout_sb.append(w)
                ycl = []
                for k in range(c.KT):
                    yl = p3y.tile([128, c.L], BF16, tag=f"ycl{k}", name=f"ycl{k}")
                    nc.sync.dma_start(
                        out=yl[:], in_=ycc_st[k * 128:(k + 1) * 128, :])
                    ycl.append(yl)
                EMT = c.EOUT // 128
                for m in range(EMT):
                    for nb in range(c.NBLK):
                        ps = p3ps.tile([128, 512], F32, tag="omm", name="omm")
                        for k in range(c.KT):
                            nc.tensor.matmul(
                                ps[:],
                                wout_sb[k][:, m * 128:(m + 1) * 128],
                                ycl[k][:, nb * 512:(nb + 1) * 512],
                                start=(k == 0), stop=(k == c.KT - 1))
                        ot = p3o.tile([128, 512], BF16, tag="oT", name="oT")
                        nc.vector.tensor_copy(ot[:], ps[:])
                        nc.sync.dma_start(
                            out=rs_in[m * 128:(m + 1) * 128,
                                      nb * 512:(nb + 1) * 512],
                            in_=ot[:])
                    # RS + residual per 2-m chunk while the next computes
                    if m % 2 == 1:
                        mc = m // 2
                        nc.gpsimd.collective_compute(
                            "ReduceScatter", ALU.add,
                            ins=[rs_in[mc * 256:(mc + 1) * 256, :]],
                            outs=[rs_out[mc * 128:(mc + 1) * 128, :]],
                            replica_groups=c.g_dh)
                        rows = slice(mc * 128, (mc + 1) * 128)
                        rsl = p4.tile([128, c.L], BF16, tag="rsl", name="rsl")
                        nc.sync.dma_start(out=rsl[:], in_=rs_out[rows, :])
                        xr = p4.tile([128, c.L], F32, tag="xr", name="xr")
                        nc.sync.dma_start(out=xr[:], in_=xres[rows, :])
                        oo = p4.tile([128, c.L], F32, tag="oo", name="oo")
                        nc.vector.tensor_tensor(oo[:], rsl[:], xr[:],
                                                op=ALU.add)
                        nc.sync.dma_start(out=out[rows, :], in_=oo[:])

    nc.compile()
    return nc


def make_core_inputs(cfg: Cfg, inputs: dict):
    """Host-side slicing of full inputs into per-core input maps."""
    c = cfg
    f = {k: np.asarray(v, dtype=np.float32) for k, v in inputs.items()}
    x = f['x']
    W = (f['in_proj_w'] * f['norm_w'][None, :]).T  # [DM, 2*DI]
    maps = []
    for core in range(c.NCORES):
        b, dr, dh = core // 4, (core // 2) % 2, core % 2
        sfx = 'f' if dr == 0 else 'b'
        dsl = slice(dh * c.DLOC, (dh + 1) * c.DLOC)
        xb = x[b] if dr == 0 else x[b][::-1]
        win = np.concatenate(
            [W[:, dsl],
             W[:, c.DI + dh * c.DLOC: c.DI + (dh + 1) * c.DLOC]], axis=1)
        esl = slice(dr * c.EOUT, (dr + 1) * c.EOUT)
        # chunked RS: device shard row r covers global e-column
        # dr*EOUT + (r//64)*128 + dh*64 + (r%64)
        r = np.arange(c.ER)
        gidx = dr * c.EOUT + (r // 128) * 256 + dh * 128 + (r % 128)
        m = {
            'x': np.ascontiguousarray(xb),
            'win_t': np.ascontiguousarray(win).astype(ml_dtypes.bfloat16),
            'wxp_t': np.ascontiguousarray(f[f'xproj_w_{sfx}'].T[dsl, :]).astype(ml_dtypes.bfloat16),
            'wdt_t': np.ascontiguousarray(f[f'dtproj_w_{sfx}'].T[:, dsl]).astype(ml_dtypes.bfloat16),
            'dtb': np.ascontiguousarray(f[f'dtproj_b_{sfx}'][dsl, None]),
            'convw': np.ascontiguousarray(f[f'conv_w_{sfx}'][dsl, 0, :]),
            'convb': np.ascontiguousarray(f[f'conv_b_{sfx}'][dsl, None]),
            'arow': np.ascontiguousarray(-np.exp(f[f'A_log_{sfx}'][0:1, :])),
            'dvec': np.ascontiguousarray(f[f'D_{sfx}'][dsl, None]),
            'wout_t': np.ascontiguousarray(0.5 * f['out_proj_w'].T[dsl, esl]).astype(ml_dtypes.bfloat16),
            'xres': np.ascontiguousarray(x[b].T[gidx, :]),
        }
        maps.append(m)
    return maps


def assemble_output(cfg: Cfg, results):
    c = cfg
    out = np.empty((c.NB, c.L, c.DM), np.float32)
    for core in range(c.NCORES):
        b, dr, dh = core // 4, (core // 2) % 2, core % 2
        r = np.arange(c.ER)
        gidx = dr * c.EOUT + (r // 128) * 256 + dh * 128 + (r % 128)
        out[b, :, gidx] = results[core]['out']
    return out


_CACHE = {}


def _get_program(cfg: Cfg):
    key = (cfg.L, cfg.DM, cfg.DI, cfg.NCORES)
    if key not in _CACHE:
        _CACHE[key] = build_program(cfg)
    return _CACHE[key]


def kernel(**inputs) -> np.ndarray:
    cfg = Cfg()
    nc = _get_program(cfg)
    in_maps = make_core_inputs(cfg, inputs)
    res = bass_utils.run_bass_kernel_spmd(
        nc, in_maps, core_ids=list(range(cfg.NCORES)))
    return assemble_output(cfg, res.results)


# revision 24
# speedup vs baseline: 1.0946x; 1.0946x over previous
"""BiMamba block Trainium2 kernel (v2).

Sharding (8 cores): core = b*4 + dir*2 + dh
  b   in {0,1}: batch element
  dir in {0,1}: scan direction (0=forward, 1=backward). Backward cores
                receive the token stream reversed by the host, so the
                device program is direction-agnostic (pure SPMD).
  dh  in {0,1}: half of d_inner (tensor-parallel over channels).

Device collectives:
  x_dbl AllReduce over dh-pairs       [[0,1],[2,3],[4,5],[6,7]]
  y AllGather over dir-pairs          [[0,2],[1,3],[4,6],[5,7]]
  out partial ReduceScatter, dh-pairs [[0,1],[2,3],[4,5],[6,7]]

Scan phase (P2) structure per core: k-tile outer, time-half middle,
state n inner.
  - B/C rows are broadcast to 128 partitions by partition-stride-0 DMA
    from DRAM (no matmul + PSUM + scalar-copy).
  - The sum over the 16 states runs on the tensor engine as
    identity-matmul accumulation into PSUM (replacing vector/gpsimd
    tensor adds).
  - The h*C product runs on gpsimd; the vector engine keeps only the
    B-mult and the scan itself.
Output partials are kept in transposed [e, t] layout end-to-end (the
ReduceScatter is elementwise, the host transposes shards), which
removes all P3 transposes.
"""

import numpy as np
import ml_dtypes

import concourse.bass as bass
import concourse.mybir as mybir
import concourse.tile as tile
from concourse import bacc, bass_utils

F32 = mybir.dt.float32
BF16 = mybir.dt.bfloat16
AF = mybir.ActivationFunctionType
ALU = mybir.AluOpType


class Cfg:
    def __init__(self, L=4096, DM=1024, DI=2048, DTR=64, DS=16, DCONV=4,
                 NB=2, EPS=1e-5):
        self.L = L          # sequence length (per stream)
        self.DM = DM        # d_model
        self.DI = DI        # d_inner
        self.DLOC = DI // 2  # channels per core
        self.DTR = DTR      # dt_rank
        self.DS = DS        # d_state
        self.DCONV = DCONV
        self.NB = NB        # batch elements
        self.EPS = EPS
        self.NXP = DTR + 2 * DS    # x_proj output dim
        self.EOUT = DM // 2        # output columns per core
        self.NCORES = NB * 4
        self.KT = self.DLOC // 128   # d-tiles per core (8)
        self.CT = DM // 128          # channel tiles of x (8)
        self.MT = 2 * self.DLOC // 128  # in_proj output tiles (16)
        self.NBLK = L // 512         # 512-token blocks (8)
        self.THL = L // 2            # scan t-half length (2048)
        self.ER = self.EOUT // 2     # output e-rows per core (256)
        # groups
        self.g_dh = [[b * 4 + d * 2, b * 4 + d * 2 + 1]
                     for b in range(NB) for d in range(2)]
        self.g_dir = [[b * 4 + dh, b * 4 + 2 + dh]
                      for b in range(NB) for dh in range(2)]


def build_program(cfg: Cfg):
    c = cfg
    nc = bacc.Bacc("TRN2", num_devices=c.NCORES)

    # ---------------- I/O ----------------
    x_in = nc.dram_tensor("x", [c.L, c.DM], F32, kind="ExternalInput")
    win_t = nc.dram_tensor("win_t", [c.DM, 2 * c.DLOC], BF16, kind="ExternalInput")
    wxp_t = nc.dram_tensor("wxp_t", [c.DLOC, c.NXP], BF16, kind="ExternalInput")
    wdt_t = nc.dram_tensor("wdt_t", [c.DTR, c.DLOC], BF16, kind="ExternalInput")
    dtb = nc.dram_tensor("dtb", [c.DLOC, 1], F32, kind="ExternalInput")
    convw = nc.dram_tensor("convw", [c.DLOC, c.DCONV], F32, kind="ExternalInput")
    convb = nc.dram_tensor("convb", [c.DLOC, 1], F32, kind="ExternalInput")
    arow = nc.dram_tensor("arow", [1, c.DS], F32, kind="ExternalInput")
    dvec = nc.dram_tensor("dvec", [c.DLOC, 1], F32, kind="ExternalInput")
    wout_t = nc.dram_tensor("wout_t", [c.DLOC, c.EOUT], BF16, kind="ExternalInput")
    xres = nc.dram_tensor("xres", [c.ER, c.L], F32, kind="ExternalInput")
    out = nc.dram_tensor("out", [c.ER, c.L], F32, kind="ExternalOutput")

    # ---------------- DRAM scratch ----------------
    xi_st = nc.dram_tensor("xi_st", [c.DLOC, c.L], F32)
    xc_st = nc.dram_tensor("xc_st", [c.DLOC, c.L], BF16)
    z_st = nc.dram_tensor("z_st", [c.DLOC, c.L], BF16)
    t1_st = nc.dram_tensor("t1_st", [c.DLOC, c.L], BF16)   # silu(z)
    xd_in = nc.dram_tensor("xd_in", [c.NXP, c.L], BF16)
    xd_out = nc.dram_tensor("xd_out", [c.NXP, c.L], BF16)
    y_in = [nc.dram_tensor(f"y_in{th}", [c.DLOC, c.THL], BF16)
            for th in range(2)]
    ycc_st = nc.dram_tensor("ycc_st", [c.DLOC, c.L], BF16)
    y_agp = [[nc.dram_tensor(f"y_agp{i}_{th}", [2 * 128, c.THL], BF16)
              for th in range(2)] for i in range(c.KT)]
    rs_in = nc.dram_tensor("rs_in", [c.EOUT, c.L], BF16)
    rs_out = nc.dram_tensor("rs_out", [c.ER, c.L], BF16)

    def rev_ap(t, n):
        """AP reading AP/tile t with the free (last) dim reversed (length n)."""
        a = t[:] if hasattr(t, 'tile_id') or not isinstance(t, bass.AP) else t
        ap = [list(d) for d in a.ap]
        assert ap[-1][0] == 1 and ap[-1][1] == n
        ap[-1] = [-1, n]
        return bass.AP(tensor=a.tensor, offset=a.offset + (n - 1), ap=ap)

    with tile.TileContext(nc) as tc:
        # ======== persistent constants ========
        with tc.tile_pool(name="wts", bufs=1) as wts:
            ident = wts.tile([128, 128], F32, tag="ident", name="ident")
            from concourse.masks import make_identity
            make_identity(nc, ident[:])
            identb = wts.tile([128, 128], BF16, tag="identb", name="identb")
            nc.vector.tensor_copy(identb[:], ident[:])
            eps_c = wts.tile([128, 1], F32, tag="eps_c", name="eps_c")
            nc.vector.memset(eps_c[:], c.EPS)

            acols = []
            for n in range(c.DS):
                acol = wts.tile([128, 1], F32, tag=f"acol{n}", name=f"acol{n}")
                nc.sync.dma_start(
                    out=acol[:],
                    in_=bass.AP(tensor=arow, offset=n, ap=[[0, 128], [1, 1]]))
                acols.append(acol)

            dtb_c, dv_c, cw_c, cb_c = [], [], [], []
            for k in range(c.KT):
                t1 = wts.tile([128, 1], F32, tag=f"dtb{k}", name=f"dtb{k}")
                nc.sync.dma_start(out=t1[:], in_=dtb[k * 128:(k + 1) * 128, :])
                dtb_c.append(t1)
                t2 = wts.tile([128, 1], F32, tag=f"dv{k}", name=f"dv{k}")
                nc.sync.dma_start(out=t2[:], in_=dvec[k * 128:(k + 1) * 128, :])
                dv_c.append(t2)
                t3 = wts.tile([128, c.DCONV], F32, tag=f"cw{k}", name=f"cw{k}")
                nc.sync.dma_start(out=t3[:], in_=convw[k * 128:(k + 1) * 128, :])
                cw_c.append(t3)
                t4 = wts.tile([128, 1], F32, tag=f"cb{k}", name=f"cb{k}")
                nc.sync.dma_start(out=t4[:], in_=convb[k * 128:(k + 1) * 128, :])
                cb_c.append(t4)

            # ======== P0: norm + transpose + in_proj ========
            with tc.tile_pool(name="p0w", bufs=1) as p0w, \
                 tc.tile_pool(name="p0", bufs=3) as p0, \
                 tc.tile_pool(name="p0t", bufs=1) as p0t, \
                 tc.tile_pool(name="p0ps", bufs=2, space="PSUM") as p0ps, \
                 tc.tile_pool(name="p0pm", bufs=4, space="PSUM") as p0pm:
                win_sb = []
                for k2 in range(c.CT):
                    w = p0w.tile([128, 2 * c.DLOC], BF16, tag=f"win{k2}", name=f"win{k2}")
                    nc.sync.dma_start(out=w[:],
                                      in_=win_t[k2 * 128:(k2 + 1) * 128, :])
                    win_sb.append(w)

                xnT_all = {}
                for tb in range(c.NBLK):
                    xnT = [p0t.tile([128, 512], BF16, tag=f"xnT{tb}_{k2}", name=f"xnT{tb}_{k2}")
                           for k2 in range(c.CT)]
                    xnT_all[tb] = xnT
                    for tt in range(4):
                        rows = slice(tb * 512 + tt * 128,
                                     tb * 512 + (tt + 1) * 128)
                        xt = p0.tile([128, c.DM], F32, tag="xt", name="xt")
                        nc.sync.dma_start(out=xt[:], in_=x_in[rows, :])
                        xsq = p0.tile([128, c.DM], F32, tag="xsq", name="xsq")
                        ssc = p0.tile([128, 1], F32, tag="ssc", name="ssc")
                        nc.scalar.activation(xsq[:], xt[:], AF.Square,
                                             accum_out=ssc[:])
                        sq = p0.tile([128, 1], F32, tag="sq", name="sq")
                        nc.scalar.activation(sq[:], ssc[:], AF.Sqrt,
                                             scale=1.0 / c.DM, bias=eps_c[:])
                        rn = p0.tile([128, 1], F32, tag="rn", name="rn")
                        nc.vector.reciprocal(rn[:], sq[:])
                        xn = p0.tile([128, c.DM], F32, tag="xn", name="xn")
                        nc.vector.tensor_scalar_mul(xn[:], xt[:], rn[:])
                        for ct4 in range(max(1, c.CT // 4)):
                            nsub = min(4, c.CT - ct4 * 4)
                            pst = p0ps.tile([128, 512], F32, tag="pst", name="pst")
                            for j in range(nsub):
                                ct = ct4 * 4 + j
                                nc.tensor.transpose(
                                    pst[:, j * 128:(j + 1) * 128],
                                    xn[:, ct * 128:(ct + 1) * 128], ident[:])
                            for j in range(nsub):
                                ct = ct4 * 4 + j
                                nc.scalar.activation(
                                    xnT[ct][:, tt * 128:(tt + 1) * 128],
                                    pst[:, j * 128:(j + 1) * 128], AF.Copy)
                for m in range(c.MT):
                    for tb in range(c.NBLK):
                        ps = p0pm.tile([128, 512], F32, tag="mm", name="mm")
                        for k2 in range(c.CT):
                            nc.tensor.matmul(
                                ps[:],
                                win_sb[k2][:, m * 128:(m + 1) * 128],
                                xnT_all[tb][k2][:],
                                start=(k2 == 0), stop=(k2 == c.CT - 1))
                        if m < c.KT:
                            dst, r0 = xi_st, m * 128
                            pcp = p0.tile([128, 512], F32, tag="pcp", name="pcp")
                        else:
                            dst, r0 = z_st, (m - c.KT) * 128
                            pcp = p0.tile([128, 512], BF16, tag="pcpb", name="pcpb")
                        nc.vector.tensor_copy(pcp[:], ps[:])
                        nc.sync.dma_start(
                            out=dst[r0:r0 + 128, tb * 512:(tb + 1) * 512],
                            in_=pcp[:])

            # ======== P1: conv + silu + x_proj partials; silu(z) prep ======
            with tc.tile_pool(name="p1", bufs=2) as p1, \
                 tc.tile_pool(name="p1ps", bufs=1, space="PSUM") as p1ps:
                xdp = [p1ps.tile([c.NXP, 512], F32, tag=f"xdp{nb}", name=f"xdp{nb}")
                       for nb in range(c.NBLK)]
                for k in range(c.KT):
                    xi = p1.tile([128, c.L], F32, tag="xi", name="xi")
                    nc.sync.dma_start(out=xi[:],
                                      in_=xi_st[k * 128:(k + 1) * 128, :])
                    cv = p1.tile([128, c.L], F32, tag="cv", name="cv")
                    nc.vector.tensor_scalar_mul(cv[:], xi[:], cw_c[k][:, 3:4])
                    for kk in (2, 1, 0):
                        sh = 3 - kk
                        nc.vector.scalar_tensor_tensor(
                            cv[:, sh:c.L], xi[:, 0:c.L - sh],
                            cw_c[k][:, kk:kk + 1],
                            cv[:, sh:c.L], ALU.mult, ALU.add)
                    nc.vector.tensor_scalar_add(cv[:], cv[:], cb_c[k][:])
                    sg = p1.tile([128, c.L], F32, tag="sg", name="sg")
                    nc.scalar.activation(sg[:], cv[:], AF.Sigmoid)
                    xcb = p1.tile([128, c.L], BF16, tag="xcb", name="xcb")
                    nc.vector.tensor_tensor(xcb[:], cv[:], sg[:], op=ALU.mult)
                    nc.sync.dma_start(out=xc_st[k * 128:(k + 1) * 128, :],
                                      in_=xcb[:])
                    wxp = p1.tile([128, c.NXP], BF16, tag="wxp", name="wxp")
                    nc.sync.dma_start(out=wxp[:],
                                      in_=wxp_t[k * 128:(k + 1) * 128, :])
                    for nb in range(c.NBLK):
                        nc.tensor.matmul(
                            xdp[nb][:], wxp[:],
                            xcb[:, nb * 512:(nb + 1) * 512],
                            start=(k == 0), stop=(k == c.KT - 1))
                for nb in range(c.NBLK):
                    xdc = p1.tile([c.NXP, 512], BF16, tag="xdc", name="xdc")
                    nc.vector.tensor_copy(xdc[:], xdp[nb][:])
                    nc.sync.dma_start(
                        out=xd_in[:, nb * 512:(nb + 1) * 512], in_=xdc[:])

            nc.gpsimd.collective_compute(
                "AllReduce", ALU.add, ins=[xd_in.ap()], outs=[xd_out.ap()],
                replica_groups=c.g_dh)

            # silu(z) gate prep — independent of the AR, fills its latency
            with tc.tile_pool(name="p1z", bufs=2) as p1z:
                for k in range(c.KT):
                    zb = p1z.tile([128, c.L], BF16, tag="zb", name="zb")
                    nc.sync.dma_start(out=zb[:],
                                      in_=z_st[k * 128:(k + 1) * 128, :])
                    sgz = p1z.tile([128, c.L], BF16, tag="sgz", name="sgz")
                    nc.scalar.activation(sgz[:], zb[:], AF.Sigmoid)
                    t1k = p1z.tile([128, c.L], BF16, tag="t1k", name="t1k")
                    nc.vector.tensor_tensor(t1k[:], sgz[:], zb[:], op=ALU.mult)
                    nc.sync.dma_start(out=t1_st[k * 128:(k + 1) * 128, :],
                                      in_=t1k[:])

            # ======== P2: dt_proj + scan core ========
            with tc.tile_pool(name="p2w", bufs=1) as p2w:
                xdbl = p2w.tile([c.DTR, c.L], BF16, tag="xdbl", name="xdbl")
                nc.sync.dma_start(out=xdbl[:], in_=xd_out[0:c.DTR, :])
                wdt = p2w.tile([c.DTR, c.DLOC], BF16, tag="wdt", name="wdt")
                nc.sync.dma_start(out=wdt[:], in_=wdt_t[:, :])

                with tc.tile_pool(name="p2k2", bufs=2) as p2k2, \
                     tc.tile_pool(name="p2k1", bufs=1) as p2k1, \
                     tc.tile_pool(name="p2r", bufs=3) as p2r, \
                     tc.tile_pool(name="p2s", bufs=2) as p2s, \
                     tc.tile_pool(name="p2h", bufs=2) as p2h, \
                     tc.tile_pool(name="p2hi", bufs=1) as p2hi, \
                     tc.tile_pool(name="p2g", bufs=1) as p2g, \
                     tc.tile_pool(name="p2dps", bufs=2, space="PSUM") as p2dps, \
                     tc.tile_pool(name="p2ya", bufs=1, space="PSUM") as p2ya:
                    hinit = [p2hi.tile([128, 1], BF16, tag=f"hi{n}", name=f"hi{n}")
                             for n in range(c.DS)]

                    def emit_an(dl, th, n):
                        an = p2s.tile([128, c.THL], F32, tag="an", name="an")
                        nc.scalar.activation(
                            an[:], dl[:, th * c.THL:(th + 1) * c.THL],
                            AF.Exp, scale=acols[n][:])
                        return an

                    def dt_chain(k):
                        """dl/du/xcd for k-tile k (software-pipelined)."""
                        dl = p2k2.tile([128, c.L], F32, tag="dl", name="dl")
                        for nb in range(c.NBLK):
                            dps = p2dps.tile([128, 512], F32, tag="dps", name="dps")
                            nc.tensor.matmul(
                                dps[:],
                                wdt[:, k * 128:(k + 1) * 128],
                                xdbl[0:c.DTR, nb * 512:(nb + 1) * 512],
                                start=True, stop=True)
                            esl = p2k1.tile([128, 512], F32, tag="esl", name="esl")
                            nc.scalar.activation(esl[:], dps[:], AF.Exp,
                                                 bias=dtb_c[k][:])
                            nc.scalar.activation(
                                dl[:, nb * 512:(nb + 1) * 512], esl[:],
                                AF.Ln, bias=1.0)
                        xcb = p2k1.tile([128, c.L], BF16, tag="xck", name="xck")
                        nc.sync.dma_start(
                            out=xcb[:], in_=xc_st[k * 128:(k + 1) * 128, :])
                        du = p2k2.tile([128, c.L], BF16, tag="du", name="du")
                        nc.vector.tensor_tensor(du[:], dl[:], xcb[:],
                                                op=ALU.mult)
                        xcd = p2k2.tile([128, c.L], BF16, tag="xcd", name="xcd")
                        nc.vector.tensor_scalar_mul(xcd[:], xcb[:], dv_c[k][:])
                        return dl, du, xcd

                    def combine_dir(k):
                        """ycc = y_f + rev(y_b) for k-tile k (both halves)."""
                        yk = p2g.tile([128, c.L], BF16, tag="ycmb", name="ycmb")
                        for th in range(2):
                            b0 = p2g.tile([128, c.THL], BF16, tag="b0", name="b0")
                            nc.sync.dma_start(out=b0[:],
                                              in_=y_agp[k][th][0:128, :])
                            b1 = p2g.tile([128, c.THL], BF16, tag="b1", name="b1")
                            nc.sync.dma_start(out=b1[:],
                                              in_=y_agp[k][1 - th][128:256, :])
                            nc.vector.tensor_tensor(
                                yk[:, th * c.THL:(th + 1) * c.THL], b0[:],
                                rev_ap(b1[:], c.THL), op=ALU.add)
                        nc.sync.dma_start(
                            out=ycc_st[k * 128:(k + 1) * 128, :], in_=yk[:])

                    cur = dt_chain(0)
                    an_next = emit_an(cur[0], 0, 0)
                    for k in range(c.KT):
                        dl, du, xcd = cur
                        ya_sb = p2k1.tile([128, c.L], BF16, tag="yasb", name="yasb")
                        t1k = p2k1.tile([128, c.L], BF16, tag="t1g", name="t1g")
                        nc.sync.dma_start(
                            out=t1k[:], in_=t1_st[k * 128:(k + 1) * 128, :])
                        for th in range(2):
                            t0 = th * c.THL
                            tsl = slice(t0, t0 + c.THL)
                            ya_ps = p2ya.tile([128, c.THL], F32, tag="ya", name="ya")
                            # init ya with D*xc (identity matmul)
                            for j in range(c.THL // 512):
                                nc.tensor.matmul(
                                    ya_ps[:, j * 512:(j + 1) * 512], identb[:],
                                    xcd[:, t0 + j * 512:t0 + (j + 1) * 512],
                                    start=True, stop=False)
                            for n in range(c.DS):
                                brow = p2r.tile([128, c.THL], BF16,
                                                tag="brow", name="brow")
                                nc.sync.dma_start(
                                    out=brow[:],
                                    in_=bass.AP(tensor=xd_out,
                                                offset=(c.DTR + n) * c.L + t0,
                                                ap=[[0, 128], [1, c.THL]]))
                                crow = p2r.tile([128, c.THL], BF16,
                                                tag="crow", name="crow")
                                nc.sync.dma_start(
                                    out=crow[:],
                                    in_=bass.AP(tensor=xd_out,
                                                offset=(c.DTR + c.DS + n) * c.L + t0,
                                                ap=[[0, 128], [1, c.THL]]))
                                bn = p2s.tile([128, c.THL], BF16, tag="bn", name="bn")
                                nc.vector.tensor_tensor(bn[:], du[:, tsl],
                                                        brow[:], op=ALU.mult)
                                an_cur = an_next
                                if n + 1 < c.DS:
                                    an_next = emit_an(dl, th, n + 1)
                                elif th == 0:
                                    an_next = emit_an(dl, 1, 0)
                                h = p2h.tile([128, c.THL], BF16, tag="h", name="h")
                                init = 0.0 if th == 0 else hinit[n][:, 0:1]
                                nc.vector.tensor_tensor_scan(
                                    h[:], an_cur[:], bn[:], init,
                                    ALU.mult, ALU.add)
                                if th == 0:
                                    nc.scalar.activation(
                                        hinit[n][:], h[:, c.THL - 1:c.THL],
                                        AF.Copy)
                                zt = p2s.tile([128, c.THL], BF16, tag="zt", name="zt")
                                nc.vector.tensor_tensor(zt[:], h[:], crow[:],
                                                        op=ALU.mult)
                                for j in range(c.THL // 512):
                                    nc.tensor.matmul(
                                        ya_ps[:, j * 512:(j + 1) * 512],
                                        identb[:],
                                        zt[:, j * 512:(j + 1) * 512],
                                        start=False, stop=(n == c.DS - 1))
                            # next k's dt chain + first an while th1 scans
                            if th == 1 and k + 1 < c.KT:
                                cur = dt_chain(k + 1)
                                an_next = emit_an(cur[0], 0, 0)
                            nc.scalar.activation(ya_sb[:, tsl], ya_ps[:],
                                                 AF.Copy)
                            # per-half gate + AllGather
                            ycg = p2s.tile([128, c.THL], BF16, tag="ycg", name="ycg")
                            nc.vector.tensor_tensor(ycg[:], t1k[:, tsl],
                                                    ya_sb[:, tsl],
                                                    op=ALU.mult)
                            nc.sync.dma_start(
                                out=y_in[th][k * 128:(k + 1) * 128, :],
                                in_=ycg[:])
                            nc.gpsimd.collective_compute(
                                "AllGather", ALU.bypass,
                                ins=[y_in[th][k * 128:(k + 1) * 128, :]],
                                outs=[y_agp[k][th].ap()],
                                replica_groups=c.g_dir)
                        # combine directions for k-2 (its AllGathers landed
                        # long ago -> no vector stall): ycc = y_f + rev(y_b)
                        if k >= 2:
                            combine_dir(k - 2)
                    combine_dir(c.KT - 2)
                    combine_dir(c.KT - 1)

            # ======== P3: out_proj (transposed layout, chunked RS) ======
            with tc.tile_pool(name="p3w", bufs=1) as p3w, \
                 tc.tile_pool(name="p3y", bufs=1) as p3y, \
                 tc.tile_pool(name="p3o", bufs=3) as p3o, \
                 tc.tile_pool(name="p4", bufs=2) as p4, \
                 tc.tile_pool(name="p3ps", bufs=4, space="PSUM") as p3ps:
                wout_sb = []
                for k in range(c.KT):
                    w = p3w.tile([128, c.EOUT], BF16, tag=f"wo{k}", name=f"wo{k}")
                    nc.sync.dma_start(out=w[:],
                                      in_=wout_t[k * 128:(k + 1) * 128, :])
                    wout_sb.append(w)
                ycl = []
                for k in range(c.KT):
                    yl = p3y.tile([128, c.L], BF16, tag=f"ycl{k}", name=f"ycl{k}")
                    nc.sync.dma_start(
                        out=yl[:], in_=ycc_st[k * 128:(k + 1) * 128, :])
                    ycl.append(yl)
                EMT = c.EOUT // 128
                for m in range(EMT):
                    for nb in range(c.NBLK):
                        ps = p3ps.tile([128, 512], F32, tag="omm", name="omm")
                        for k in range(c.KT):
                            nc.tensor.matmul(
                                ps[:],
                                wout_sb[k][:, m * 128:(m + 1) * 128],
                                ycl[k][:, nb * 512:(nb + 1) * 512],
                                start=(k == 0), stop=(k == c.KT - 1))
                        ot = p3o.tile([128, 512], BF16, tag="oT", name="oT")
                        nc.vector.tensor_copy(ot[:], ps[:])
                        nc.sync.dma_start(
                            out=rs_in[m * 128:(m + 1) * 128,
                                      nb * 512:(nb + 1) * 512],
                            in_=ot[:])
                    # RS + residual per 2-m chunk while the next computes
                    if m % 2 == 1:
                        mc = m // 2
                        nc.gpsimd.collective_compute(
                            "ReduceScatter", ALU.add,
                            ins=[rs_in[mc * 256:(mc + 1) * 256, :]],
                            outs=[rs_out[mc * 128:(mc + 1) * 128, :]],
                            replica_groups=c.g_dh)
                        rows = slice(mc * 128, (mc + 1) * 128)
                        rsl = p4.tile([128, c.L], BF16, tag="rsl", name="rsl")
                        nc.sync.dma_start(out=rsl[:], in_=rs_out[rows, :])
                        xr = p4.tile([128, c.L], F32, tag="xr", name="xr")
                        nc.sync.dma_start(out=xr[:], in_=xres[rows, :])
                        oo = p4.tile([128, c.L], F32, tag="oo", name="oo")
                        nc.vector.tensor_tensor(oo[:], rsl[:], xr[:],
                                                op=ALU.add)
                        nc.sync.dma_start(out=out[rows, :], in_=oo[:])

    nc.compile()
    return nc


def make_core_inputs(cfg: Cfg, inputs: dict):
    """Host-side slicing of full inputs into per-core input maps."""
    c = cfg
    f = {k: np.asarray(v, dtype=np.float32) for k, v in inputs.items()}
    x = f['x']
    W = (f['in_proj_w'] * f['norm_w'][None, :]).T  # [DM, 2*DI]
    maps = []
    for core in range(c.NCORES):
        b, dr, dh = core // 4, (core // 2) % 2, core % 2
        sfx = 'f' if dr == 0 else 'b'
        dsl = slice(dh * c.DLOC, (dh + 1) * c.DLOC)
        xb = x[b] if dr == 0 else x[b][::-1]
        win = np.concatenate(
            [W[:, dsl],
             W[:, c.DI + dh * c.DLOC: c.DI + (dh + 1) * c.DLOC]], axis=1)
        esl = slice(dr * c.EOUT, (dr + 1) * c.EOUT)
        # chunked RS: device shard row r covers global e-column
        # dr*EOUT + (r//64)*128 + dh*64 + (r%64)
        r = np.arange(c.ER)
        gidx = dr * c.EOUT + (r // 128) * 256 + dh * 128 + (r % 128)
        m = {
            'x': np.ascontiguousarray(xb),
            'win_t': np.ascontiguousarray(win).astype(ml_dtypes.bfloat16),
            'wxp_t': np.ascontiguousarray(f[f'xproj_w_{sfx}'].T[dsl, :]).astype(ml_dtypes.bfloat16),
            'wdt_t': np.ascontiguousarray(f[f'dtproj_w_{sfx}'].T[:, dsl]).astype(ml_dtypes.bfloat16),
            'dtb': np.ascontiguousarray(f[f'dtproj_b_{sfx}'][dsl, None]),
            'convw': np.ascontiguousarray(f[f'conv_w_{sfx}'][dsl, 0, :]),
            'convb': np.ascontiguousarray(f[f'conv_b_{sfx}'][dsl, None]),
            'arow': np.ascontiguousarray(-np.exp(f[f'A_log_{sfx}'][0:1, :])),
            'dvec': np.ascontiguousarray(f[f'D_{sfx}'][dsl, None]),
            'wout_t': np.ascontiguousarray(0.5 * f['out_proj_w'].T[dsl, esl]).astype(ml_dtypes.bfloat16),
            'xres': np.ascontiguousarray(x[b].T[gidx, :]),
        }
        maps.append(m)
    return maps


def assemble_output(cfg: Cfg, results):
    c = cfg
    out = np.empty((c.NB, c.L, c.DM), np.float32)
    for core in range(c.NCORES):
        b, dr, dh = core // 4, (core // 2) % 2, core % 2
        r = np.arange(c.ER)
        gidx = dr * c.EOUT + (r // 128) * 256 + dh * 128 + (r % 128)
        out[b, :, gidx] = results[core]['out']
    return out


_CACHE = {}


def _get_program(cfg: Cfg):
    key = (cfg.L, cfg.DM, cfg.DI, cfg.NCORES)
    if key not in _CACHE:
        _CACHE[key] = build_program(cfg)
    return _CACHE[key]


def kernel(**inputs) -> np.ndarray:
    cfg = Cfg()
    nc = _get_program(cfg)
    in_maps = make_core_inputs(cfg, inputs)
    res = bass_utils.run_bass_kernel_spmd(
        nc, in_maps, core_ids=list(range(cfg.NCORES)))
    return assemble_output(cfg, res.results)
